# revision 10
# baseline (speedup 1.0000x reference)
"""Distributed Trainium2 Bass kernel for the ACMProxy loss.

Sharding: proxy bank (N=65536) split across 8 NeuronCores, camera-grouped and
evenly dealt so all cores share one SPMD graph. Device does the heavy
(B=64)x(N/8) matmul (bf16 inputs, f32 PSUM) + a max8-based extraction
epilogue; host merges tiny per-core candidate lists exactly (f64 logsumexp,
top-50 of per-chunk top-16s) and computes the B=64 batch terms (MMD/triplet)
in numpy.

Device outputs per core (all f32, packed rows p = 64*half + batch_row):
  o_pos (128, njt*8) : per-(chunk) top-8 of d + pen_pos  -> all pos d values
  o_neg (128, njt*16): per-(chunk) top-16 of d + pen_neg -> neg candidates
  o_cam (128, 24)    : per-camera top-8 of d (camera-contiguous slots)
"""

import ml_dtypes
import numpy as np

import concourse.mybir as mybir
from concourse import bacc
from concourse.tile import TileContext
from concourse.bass_utils import run_bass_kernel_spmd

# problem constants (hardcoded)
B, D, N = 64, 2048, 65536
M = 8
TEMP = 0.07
NUM_HARDS = 50
LAM_DIS = 0.05
LAM_INS = 0.05
GAMMA = 0.9
NK = 5
MAX_CAMS = 8
NCAMS = 6

PEN = 1.0e5     # exclusion penalty (bf16 -> -99840); real d values are in [-8, 8]
NEG_ROUNDS = 2  # top-16 kept per 512-chunk (max observed top50-members/chunk = 5)
JT = 512

_cache = {}


# ---------------------------------------------------------------- layout plan
def _plan(cids):
    idx_by_cam = [np.nonzero(cids == c)[0] for c in range(NCAMS)]
    percore = [[idx_by_cam[c][m::M] for c in range(NCAMS)] for m in range(M)]
    slot = [max(len(percore[m][c]) for m in range(M)) for c in range(NCAMS)]
    wA, wB = sum(slot[0:3]), sum(slot[3:6])
    W = ((max(wA, wB) + 7) // 8) * 8
    offs = {}
    o = 0
    for c in range(3):
        offs[c] = o
        o += slot[c]
    o = 0
    for c in range(3, 6):
        offs[c] = o
        o += slot[c]
    njt = (W + JT - 1) // JT
    return percore, slot, offs, W, njt


def _prep_core(m, percore, slot, offs, W, proxy, targets, cams, pids):
    col_g = np.full(2 * W, -1, dtype=np.int64)
    cid_col = np.full(2 * W, -1, dtype=np.int64)
    for c in range(NCAMS):
        h = 0 if c < 3 else 1
        base = h * W + offs[c]
        g = percore[m][c]
        col_g[base:base + len(g)] = g
        cid_col[base:base + slot[c]] = c

    real = col_g >= 0
    proxT = np.zeros((D, 2 * W), dtype=np.float32)
    proxT[:, real] = proxy[col_g[real], :].T
    # repack jtile-contiguous bf16: per partition p, per jtile j: [h][k][c]
    KT = D // 128
    A = proxT.reshape(KT, 128, 2, W)
    parts = []
    for j in range((W + JT - 1) // JT):
        w = min(JT, W - j * JT)
        blk = A[:, :, :, j * JT:j * JT + w]          # (KT,128,2,w)
        parts.append(np.transpose(blk, (1, 2, 0, 3)).reshape(128, 2 * KT * w))
    proxP = np.ascontiguousarray(np.concatenate(parts, axis=1).astype(ml_dtypes.bfloat16))
    pid_col = np.where(real, pids[np.where(real, col_g, 0)], -1)

    ppos = np.zeros((128, W), dtype=ml_dtypes.bfloat16)
    pneg = np.zeros((128, W), dtype=ml_dtypes.bfloat16)
    njt = (W + JT - 1) // JT
    nposh = np.zeros((128, njt), dtype=np.int64)
    for h in range(2):
        cols = slice(h * W, (h + 1) * W)
        pm = (targets[:, None] == pid_col[None, cols]) & (cams[:, None] != cid_col[None, cols]) & real[None, cols]
        nm = (targets[:, None] != pid_col[None, cols]) & real[None, cols]
        ppos[64 * h:64 * h + 64] = np.where(pm, 0.0, -PEN)
        pneg[64 * h:64 * h + 64] = np.where(nm, 0.0, -PEN)
        for j in range(njt):
            nposh[64 * h:64 * h + 64, j] = pm[:, j * JT:(j + 1) * JT].sum(1)
    return {"proxP": proxP, "ppos": ppos, "pneg": pneg}, nposh


# ---------------------------------------------------------------- bass kernel
def _build(W, njt, slot, offs):
    KT = D // 128
    f32 = mybir.dt.float32
    bf16 = mybir.dt.bfloat16
    nc = bacc.Bacc("TRN2", target_bir_lowering=False, debug=False, num_devices=M)

    proxP_e = nc.dram_tensor("proxP", [128, 2 * KT * W], bf16, kind="ExternalInput").ap()
    ppos_e = nc.dram_tensor("ppos", [128, W], bf16, kind="ExternalInput").ap()
    pneg_e = nc.dram_tensor("pneg", [128, W], bf16, kind="ExternalInput").ap()
    xP_e = nc.dram_tensor("xP", [128, KT * B], bf16, kind="ExternalInput").ap()

    o_pos = nc.dram_tensor("o_pos", [128, njt * 8], f32, kind="ExternalOutput").ap()
    o_neg = nc.dram_tensor("o_neg", [128, njt * 8 * NEG_ROUNDS], f32, kind="ExternalOutput").ap()
    o_cam = nc.dram_tensor("o_cam", [128, 24], f32, kind="ExternalOutput").ap()

    with TileContext(nc) as tc:
        with (
            tc.tile_pool(name="const", bufs=1) as constp,
            tc.tile_pool(name="rhs", bufs=4) as rhsp,
            tc.tile_pool(name="ps", bufs=2, space="PSUM") as psump,
            tc.tile_pool(name="scr", bufs=2) as scrp,
            tc.tile_pool(name="full", bufs=1) as fullp,
        ):
            xts = constp.tile([128, KT * B], bf16)
            nc.sync.dma_start(out=xts[:], in_=xP_e[:, :])
            ppos_s = fullp.tile([128, W], bf16)
            pneg_s = fullp.tile([128, W], bf16)

            d_full = fullp.tile([128, W], f32)
            postop = fullp.tile([128, njt * 8], f32)
            negtop = fullp.tile([128, njt * 8 * NEG_ROUNDS], f32)
            camtop = fullp.tile([128, 24], f32)

            poff = 0
            for j in range(njt):
                w = min(JT, W - j * JT)
                jsl = slice(j * JT, j * JT + w)
                # per-jtile mask slices (spread the mask DMA through the pipe)
                nc.sync.dma_start(out=ppos_s[:, jsl], in_=ppos_e[:, jsl])
                nc.sync.dma_start(out=pneg_s[:, jsl], in_=pneg_e[:, jsl])
                psA = psump.tile([128, JT], f32, tag="psA")
                psB = psump.tile([128, JT], f32, tag="psB")
                for h in range(2):
                    rhs = rhsp.tile([128, KT * JT], bf16, tag="rhs")
                    nch = 4
                    half = KT // nch
                    for ci in range(nch):
                        nc.sync.dma_start(
                            out=rhs[:, ci * half * w:(ci + 1) * half * w],
                            in_=proxP_e[:, poff + (h * KT + ci * half) * w:
                                        poff + (h * KT + (ci + 1) * half) * w])
                    for k in range(KT):
                        xk = xts[:, k * B:(k + 1) * B]
                        rk = rhs[:, k * w:(k + 1) * w]
                        if h == 0:
                            nc.tensor.matmul(psA[0:64, :w], xk, rk,
                                             start=(k == 0), stop=(k == KT - 1))
                        else:
                            nc.tensor.matmul(psB[64:128, :w], xk, rk,
                                             start=(k == 0), stop=(k == KT - 1))
                dj = d_full[:, jsl]
                nc.scalar.copy(d_full[0:64, jsl], psA[0:64, :w])
                nc.scalar.copy(d_full[64:128, jsl], psB[64:128, :w])
                # pos candidates: top-8 of d + pen_pos
                tp = scrp.tile([128, JT], f32, tag="tp")
                nc.vector.tensor_add(tp[:, :w], dj, ppos_s[:, jsl])
                nc.vector.max(postop[:, j * 8:(j + 1) * 8], tp[:, :w])
                # neg candidates: top-16 of d + pen_neg
                mn = scrp.tile([128, JT], f32, tag="mn")
                nc.vector.tensor_add(mn[:, :w], dj, pneg_s[:, jsl])
                for r in range(NEG_ROUNDS):
                    t8 = negtop[:, (j * NEG_ROUNDS + r) * 8:(j * NEG_ROUNDS + r) * 8 + 8]
                    nc.vector.max(t8, mn[:, :w])
                    if r < NEG_ROUNDS - 1:
                        nc.vector.match_replace(mn[:, :w], t8, mn[:, :w], -1e9)
                nc.sync.dma_start(out=o_pos[:, j * 8:(j + 1) * 8],
                                  in_=postop[:, j * 8:(j + 1) * 8])
                nc.sync.dma_start(
                    out=o_neg[:, j * NEG_ROUNDS * 8:(j + 1) * NEG_ROUNDS * 8],
                    in_=negtop[:, j * NEG_ROUNDS * 8:(j + 1) * NEG_ROUNDS * 8])
                # camera top8s whose slot finished inside this jtile
                for c in range(NCAMS):
                    h = 0 if c < 3 else 1
                    end = offs[c] + slot[c]
                    if j * JT < end <= min((j + 1) * JT, W):
                        pr = slice(64 * h, 64 * h + 64)
                        cw = c if c < 3 else c - 3
                        nc.vector.max(camtop[pr, cw * 8:cw * 8 + 8],
                                      d_full[pr, offs[c]:end])
                poff += 2 * KT * w

            nc.sync.dma_start(out=o_cam[:, :], in_=camtop[:])

    nc.compile()
    return nc


# ---------------------------------------------------------------- host math
def _host_batch_terms(x, targets, cams, cids_hist, vals, D_cam):
    """Mirror of reference _acm_dis / _acm_ins with device-supplied `vals`."""
    Bsz = x.shape[0]
    C = MAX_CAMS
    f32 = np.float32

    diff = x[:, None, :] - x[None, :, :]
    d2 = np.sum(diff * diff, axis=-1, dtype=f32)
    eye = np.eye(Bsz, dtype=bool)
    pw = np.sqrt(np.where(eye, f32(1.0), d2)).astype(f32) * (~eye)

    # ---- _acm_dis (MMD between intra/inter camera pair distances)
    iu, ju = np.triu_indices(Bsz, 1)
    dvec = pw[iu, ju].astype(f32)
    same = cams[iu] == cams[ju]
    wx = same.astype(f32)
    wy = (~same).astype(f32)
    n = wx.sum(dtype=f32)
    mm = wy.sum(dtype=f32)
    sq = (dvec[:, None] - dvec[None, :]) ** 2
    Sxx = wx @ sq @ wx
    Syy = wy @ sq @ wy
    denom = max(f32(1.0), n * n - n + mm * mm - mm)
    sigma = max(max(Sxx + Syy, f32(1e-6)) / denom, f32(1e-6))
    K = np.exp(-sq / sigma, dtype=f32)
    kxx = (wx @ K @ wx) / max(n * n, f32(1.0))
    kyy = (wy @ K @ wy) / max(mm * mm, f32(1.0))
    kxy = (wx @ K @ wy) / max(n * mm, f32(1.0))
    dis = (kxx + kyy - 2.0 * kxy) if (n >= 2 and mm >= 2) else f32(0.0)

    # ---- _acm_ins
    Moh = np.zeros((Bsz, C), dtype=f32)
    Moh[np.arange(Bsz), cams] = 1.0
    triu = np.triu(np.ones((Bsz, Bsz), dtype=f32), 1)
    pwt = pw * triu
    intra_sum = np.einsum('ic,jc,ij->c', Moh, Moh, pwt).astype(f32)
    intra_cnt = np.einsum('ic,jc,ij->c', Moh, Moh, triu).astype(f32)
    intra_mean = intra_sum / np.maximum(intra_cnt, 1.0)
    cam_cnt = Moh.sum(0)
    proxy_cnt = cids_hist.astype(f32)
    with np.errstate(invalid='ignore'):
        mean_d = (Moh.T @ vals) / np.maximum(cam_cnt, 1.0)[:, None]

    Dc = D_cam.astype(f32).copy()
    rng = np.arange(C)
    diag = np.diagonal(Dc).copy()
    diag_new = GAMMA * diag + (1.0 - GAMMA) * np.maximum(intra_mean, 1e-6)
    Dc[rng, rng] = np.where(intra_cnt >= 1.0, diag_new, diag)
    present = cam_cnt > 0
    off_mask = present[:, None] & present[None, :] & (proxy_cnt[None, :] > 0) & (~np.eye(C, dtype=bool))
    with np.errstate(invalid='ignore'):
        upd = GAMMA * Dc + (1.0 - GAMMA) * np.maximum(mean_d, 1e-6)
    Dc = np.where(off_mask, upd, Dc)
    Dc = np.maximum(Dc, 1e-6)

    dist_raw = np.maximum(1.0 - x @ x.T, 1e-8).astype(f32)
    pos_m = (targets[:, None] == targets[None, :]) & (cams[:, None] != cams[None, :])
    neg_m = targets[:, None] != targets[None, :]
    hard_pos = np.argmin(np.where(pos_m, dist_raw, np.inf), axis=1)
    hard_neg = np.argmax(np.where(neg_m, dist_raw, -np.inf), axis=1)
    idx = np.arange(Bsz)
    ic = cams
    scale_pos = np.clip(Dc[ic, ic] / Dc[ic, cams[hard_pos]], 0.1, 10.0)
    scale_neg = np.clip(Dc[ic, ic] / Dc[ic, cams[hard_neg]], 0.1, 10.0)
    d_pos = dist_raw[idx, hard_pos] * scale_pos
    d_neg = dist_raw[idx, hard_neg] * scale_neg
    trip = np.maximum(d_pos - d_neg + 0.2, 0.0)
    valid = (pos_m.sum(1) > 0) & (neg_m.sum(1) > 0)
    cnt = f32(valid.sum())
    ins = np.where(valid, trip, 0.0).sum(dtype=f32) / max(cnt, f32(1.0)) if cnt > 0 else f32(0.0)
    return f32(dis), f32(ins)


# ---------------------------------------------------------------- entry point
def kernel(**inputs):
    inp = np.asarray(inputs["inputs"], dtype=np.float32)
    targets = np.asarray(inputs["targets"]).astype(np.int64)
    cams = np.asarray(inputs["cams"]).astype(np.int64)
    proxy = np.asarray(inputs["proxy"], dtype=np.float32)
    pids = np.asarray(inputs["pids"]).astype(np.int64)
    cids = np.asarray(inputs["cids"]).astype(np.int64)
    D_cam = np.asarray(inputs["D_cam"], dtype=np.float32)

    x = inp / np.maximum(np.linalg.norm(inp, axis=1, keepdims=True), 1e-12)
    x = x.astype(np.float32)
    xP = np.ascontiguousarray(
        x.T.reshape(D // 128, 128, B).transpose(1, 0, 2)
        .reshape(128, (D // 128) * B).astype(ml_dtypes.bfloat16))

    percore, slot, offs, W, njt = _plan(cids)

    key = (W, njt, tuple(slot))
    if key not in _cache:
        _cache[key] = _build(W, njt, slot, offs)
    nc = _cache[key]

    in_maps = []
    nposh_all = []
    for m in range(M):
        im, nposh = _prep_core(m, percore, slot, offs, W, proxy, targets, cams, pids)
        im["xP"] = xP
        in_maps.append(im)
        nposh_all.append(nposh)

    res = run_bass_kernel_spmd(nc, in_maps, core_ids=list(range(M)))
    outs = res.results

    # ---------------- merge main loss ----------------
    npos = np.zeros(B, dtype=np.int64)
    pos_cands = [[] for _ in range(B)]
    neg_cands = []
    cam_cands = [[] for _ in range(NCAMS)]
    for m in range(M):
        nposh = nposh_all[m]
        npos += nposh[0:64].sum(axis=1) + nposh[64:128].sum(axis=1)
        po = outs[m]["o_pos"]  # (128, njt*8)
        pos_pack = np.concatenate([po[0:64], po[64:128]], axis=1)
        for i in range(B):
            v = pos_pack[i]
            pos_cands[i].append(v[v > -50.0])  # pen'd values are <= -9e4
        negm = outs[m]["o_neg"]
        neg_cands.append(np.concatenate([negm[0:64], negm[64:128]], axis=1))
        camv = outs[m]["o_cam"]
        for c in range(NCAMS):
            h, cw = (0, c) if c < 3 else (1, c - 3)
            cam_cands[c].append(camv[64 * h:64 * h + 64, cw * 8:cw * 8 + 8])

    lse_pos = np.full(B, -np.inf)
    mean_pos = np.zeros(B)
    for i in range(B):
        v = np.concatenate(pos_cands[i]).astype(np.float64)
        assert len(v) == npos[i], (i, len(v), npos[i])
        if len(v):
            t = v / TEMP
            tm = t.max()
            lse_pos[i] = tm + np.log(np.exp(t - tm).sum())
            mean_pos[i] = t.mean()

    allneg = np.concatenate(neg_cands, axis=1)
    top50 = -np.partition(-allneg, NUM_HARDS - 1, axis=1)[:, :NUM_HARDS]
    t = top50 / TEMP
    tm = t.max(axis=1)
    lse_neg = tm + np.log(np.sum(np.exp(t - tm[:, None]), axis=1))

    lse = np.logaddexp(lse_pos, lse_neg)
    row = np.where(npos > 0, lse - mean_pos, 0.0)
    loss = row.sum() / B

    # ---------------- batch terms ----------------
    present = np.zeros(MAX_CAMS)
    np.add.at(present, cams, 1.0)
    multi_cam = (present > 0).sum() >= 2

    if multi_cam:
        vals = np.full((B, MAX_CAMS), np.inf, dtype=np.float32)
        for c in range(NCAMS):
            cand = np.concatenate(cam_cands[c], axis=1)  # (64, M*8) d units
            top5 = -np.partition(-cand, NK - 1, axis=1)[:, :NK]
            vals[:, c] = np.maximum(1.0 - top5, 1e-8).mean(axis=1)
        cids_hist = np.zeros(MAX_CAMS)
        np.add.at(cids_hist, cids, 1.0)
        dis, ins = _host_batch_terms(x, targets, cams, cids_hist, vals, D_cam)
        loss = loss + LAM_DIS * float(dis) + LAM_INS * float(ins)

    return np.float32(loss)


# revision 11
# speedup vs baseline: 1.3197x; 1.3197x over previous
"""Distributed Trainium2 Bass kernel for the ACMProxy loss.

Sharding: proxy bank (N=65536) split across 8 NeuronCores, camera-grouped and
evenly dealt so all cores share one SPMD graph. Device does the heavy
(B=64)x(N/8) matmul (bf16 inputs, f32 PSUM) + a max8-based extraction
epilogue; host merges tiny per-core candidate lists exactly (f64 logsumexp,
top-50 of per-chunk top-16s) and computes the B=64 batch terms (MMD/triplet)
in numpy.

Device outputs per core (all f32, packed rows p = 64*half + batch_row):
  o_pos (128, njt*8) : per-(chunk) top-8 of d + pen_pos  -> all pos d values
  o_neg (128, njt*16): per-(chunk) top-16 of d + pen_neg -> neg candidates
  o_cam (128, 24)    : per-camera top-8 of d (camera-contiguous slots)
"""

import ml_dtypes
import numpy as np

import concourse.mybir as mybir
from concourse import bacc
from concourse.tile import TileContext
from concourse.bass_utils import run_bass_kernel_spmd

# problem constants (hardcoded)
B, D, N = 64, 2048, 65536
M = 8
TEMP = 0.07
NUM_HARDS = 50
LAM_DIS = 0.05
LAM_INS = 0.05
GAMMA = 0.9
NK = 5
MAX_CAMS = 8
NCAMS = 6

PEN = 1.0e5     # exclusion penalty (bf16 -> -99840); real d values are in [-8, 8]
NEG_ROUNDS = 2  # top-16 kept per 512-chunk (max observed top50-members/chunk = 5)
JT = 512

_cache = {}


# ---------------------------------------------------------------- layout plan
def _plan(cids):
    idx_by_cam = [np.nonzero(cids == c)[0] for c in range(NCAMS)]
    percore = [[idx_by_cam[c][m::M] for c in range(NCAMS)] for m in range(M)]
    slot = [max(len(percore[m][c]) for m in range(M)) for c in range(NCAMS)]
    wA, wB = sum(slot[0:3]), sum(slot[3:6])
    W = ((max(wA, wB) + 7) // 8) * 8
    offs = {}
    o = 0
    for c in range(3):
        offs[c] = o
        o += slot[c]
    o = 0
    for c in range(3, 6):
        offs[c] = o
        o += slot[c]
    njt = (W + JT - 1) // JT
    return percore, slot, offs, W, njt


def _prep_core(m, percore, slot, offs, W, proxy, targets, cams, pids):
    col_g = np.full(2 * W, -1, dtype=np.int64)
    cid_col = np.full(2 * W, -1, dtype=np.int64)
    for c in range(NCAMS):
        h = 0 if c < 3 else 1
        base = h * W + offs[c]
        g = percore[m][c]
        col_g[base:base + len(g)] = g
        cid_col[base:base + slot[c]] = c

    real = col_g >= 0
    proxT = np.zeros((D, 2 * W), dtype=np.float32)
    proxT[:, real] = proxy[col_g[real], :].T
    # repack jtile-contiguous bf16: per partition p, per jtile j: [h][k][c]
    KT = D // 128
    A = proxT.reshape(KT, 128, 2, W)
    parts = []
    for j in range((W + JT - 1) // JT):
        w = min(JT, W - j * JT)
        blk = A[:, :, :, j * JT:j * JT + w]          # (KT,128,2,w)
        parts.append(np.transpose(blk, (1, 2, 0, 3)).reshape(128, 2 * KT * w))
    proxP = np.ascontiguousarray(np.concatenate(parts, axis=1).astype(ml_dtypes.bfloat16))
    pid_col = np.where(real, pids[np.where(real, col_g, 0)], -1)

    ppos = np.zeros((128, W), dtype=ml_dtypes.bfloat16)
    pneg = np.zeros((128, W), dtype=ml_dtypes.bfloat16)
    njt = (W + JT - 1) // JT
    nposh = np.zeros((128, njt), dtype=np.int64)
    for h in range(2):
        cols = slice(h * W, (h + 1) * W)
        pm = (targets[:, None] == pid_col[None, cols]) & (cams[:, None] != cid_col[None, cols]) & real[None, cols]
        nm = (targets[:, None] != pid_col[None, cols]) & real[None, cols]
        ppos[64 * h:64 * h + 64] = np.where(pm, 0.0, -PEN)
        pneg[64 * h:64 * h + 64] = np.where(nm, 0.0, -PEN)
        for j in range(njt):
            nposh[64 * h:64 * h + 64, j] = pm[:, j * JT:(j + 1) * JT].sum(1)
    return {"proxP": proxP, "ppos": ppos, "pneg": pneg}, nposh


# ---------------------------------------------------------------- bass kernel
def _build(W, njt, slot, offs):
    KT = D // 128
    f32 = mybir.dt.float32
    bf16 = mybir.dt.bfloat16
    nc = bacc.Bacc("TRN2", target_bir_lowering=False, debug=False, num_devices=M)

    proxP_e = nc.dram_tensor("proxP", [128, 2 * KT * W], bf16, kind="ExternalInput").ap()
    ppos_e = nc.dram_tensor("ppos", [128, W], bf16, kind="ExternalInput").ap()
    pneg_e = nc.dram_tensor("pneg", [128, W], bf16, kind="ExternalInput").ap()
    xP_e = nc.dram_tensor("xP", [128, KT * B], bf16, kind="ExternalInput").ap()

    o_pos = nc.dram_tensor("o_pos", [128, njt * 8], f32, kind="ExternalOutput").ap()
    o_neg = nc.dram_tensor("o_neg", [128, njt * 8 * NEG_ROUNDS], f32, kind="ExternalOutput").ap()
    o_cam = nc.dram_tensor("o_cam", [128, 24], f32, kind="ExternalOutput").ap()

    with TileContext(nc) as tc:
        with (
            tc.tile_pool(name="const", bufs=1) as constp,
            tc.tile_pool(name="rhs", bufs=4) as rhsp,
            tc.tile_pool(name="ps", bufs=2, space="PSUM") as psump,
            tc.tile_pool(name="scr", bufs=2) as scrp,
            tc.tile_pool(name="full", bufs=1) as fullp,
        ):
            xts = constp.tile([128, KT * B], bf16)
            nc.sync.dma_start(out=xts[:], in_=xP_e[:, :])
            ppos_s = fullp.tile([128, W], bf16)
            pneg_s = fullp.tile([128, W], bf16)

            d_full = fullp.tile([128, W], f32)
            postop = fullp.tile([128, njt * 8], f32)
            negtop = fullp.tile([128, njt * 8 * NEG_ROUNDS], f32)
            camtop = fullp.tile([128, 24], f32)

            poff = 0
            for j in range(njt):
                w = min(JT, W - j * JT)
                jsl = slice(j * JT, j * JT + w)
                # per-jtile mask slices (spread the mask DMA through the pipe)
                nc.sync.dma_start(out=ppos_s[:, jsl], in_=ppos_e[:, jsl])
                nc.sync.dma_start(out=pneg_s[:, jsl], in_=pneg_e[:, jsl])
                psA = psump.tile([128, JT], f32, tag="psA")
                psB = psump.tile([128, JT], f32, tag="psB")
                for h in range(2):
                    rhs = rhsp.tile([128, KT * JT], bf16, tag="rhs")
                    nch = 4 if j == 0 else 2
                    half = KT // nch
                    for ci in range(nch):
                        nc.sync.dma_start(
                            out=rhs[:, ci * half * w:(ci + 1) * half * w],
                            in_=proxP_e[:, poff + (h * KT + ci * half) * w:
                                        poff + (h * KT + (ci + 1) * half) * w])
                    for k in range(KT):
                        xk = xts[:, k * B:(k + 1) * B]
                        rk = rhs[:, k * w:(k + 1) * w]
                        if h == 0:
                            nc.tensor.matmul(psA[0:64, :w], xk, rk,
                                             start=(k == 0), stop=(k == KT - 1))
                        else:
                            nc.tensor.matmul(psB[64:128, :w], xk, rk,
                                             start=(k == 0), stop=(k == KT - 1))
                dj = d_full[:, jsl]
                nc.scalar.copy(d_full[0:64, jsl], psA[0:64, :w])
                nc.scalar.copy(d_full[64:128, jsl], psB[64:128, :w])
                # pos candidates: top-8 of d + pen_pos
                tp = scrp.tile([128, JT], f32, tag="tp")
                nc.vector.tensor_add(tp[:, :w], dj, ppos_s[:, jsl])
                nc.vector.max(postop[:, j * 8:(j + 1) * 8], tp[:, :w])
                # neg candidates: top-16 of d + pen_neg
                mn = scrp.tile([128, JT], f32, tag="mn")
                nc.vector.tensor_add(mn[:, :w], dj, pneg_s[:, jsl])
                for r in range(NEG_ROUNDS):
                    t8 = negtop[:, (j * NEG_ROUNDS + r) * 8:(j * NEG_ROUNDS + r) * 8 + 8]
                    nc.vector.max(t8, mn[:, :w])
                    if r < NEG_ROUNDS - 1:
                        nc.vector.match_replace(mn[:, :w], t8, mn[:, :w], -1e9)
                poff += 2 * KT * w

            for c in range(NCAMS):
                h, cw = (0, c) if c < 3 else (1, c - 3)
                pr = slice(64 * h, 64 * h + 64)
                nc.vector.max(camtop[pr, cw * 8:cw * 8 + 8],
                              d_full[pr, offs[c]:offs[c] + slot[c]])
            nc.sync.dma_start(out=o_pos[:, :], in_=postop[:])
            nc.sync.dma_start(out=o_neg[:, :], in_=negtop[:])
            nc.sync.dma_start(out=o_cam[:, :], in_=camtop[:])

    nc.compile()
    return nc


# ---------------------------------------------------------------- host math
def _host_batch_terms(x, targets, cams, cids_hist, vals, D_cam):
    """Mirror of reference _acm_dis / _acm_ins with device-supplied `vals`."""
    Bsz = x.shape[0]
    C = MAX_CAMS
    f32 = np.float32

    diff = x[:, None, :] - x[None, :, :]
    d2 = np.sum(diff * diff, axis=-1, dtype=f32)
    eye = np.eye(Bsz, dtype=bool)
    pw = np.sqrt(np.where(eye, f32(1.0), d2)).astype(f32) * (~eye)

    # ---- _acm_dis (MMD between intra/inter camera pair distances)
    iu, ju = np.triu_indices(Bsz, 1)
    dvec = pw[iu, ju].astype(f32)
    same = cams[iu] == cams[ju]
    wx = same.astype(f32)
    wy = (~same).astype(f32)
    n = wx.sum(dtype=f32)
    mm = wy.sum(dtype=f32)
    sq = (dvec[:, None] - dvec[None, :]) ** 2
    Sxx = wx @ sq @ wx
    Syy = wy @ sq @ wy
    denom = max(f32(1.0), n * n - n + mm * mm - mm)
    sigma = max(max(Sxx + Syy, f32(1e-6)) / denom, f32(1e-6))
    K = np.exp(-sq / sigma, dtype=f32)
    kxx = (wx @ K @ wx) / max(n * n, f32(1.0))
    kyy = (wy @ K @ wy) / max(mm * mm, f32(1.0))
    kxy = (wx @ K @ wy) / max(n * mm, f32(1.0))
    dis = (kxx + kyy - 2.0 * kxy) if (n >= 2 and mm >= 2) else f32(0.0)

    # ---- _acm_ins
    Moh = np.zeros((Bsz, C), dtype=f32)
    Moh[np.arange(Bsz), cams] = 1.0
    triu = np.triu(np.ones((Bsz, Bsz), dtype=f32), 1)
    pwt = pw * triu
    intra_sum = np.einsum('ic,jc,ij->c', Moh, Moh, pwt).astype(f32)
    intra_cnt = np.einsum('ic,jc,ij->c', Moh, Moh, triu).astype(f32)
    intra_mean = intra_sum / np.maximum(intra_cnt, 1.0)
    cam_cnt = Moh.sum(0)
    proxy_cnt = cids_hist.astype(f32)
    with np.errstate(invalid='ignore'):
        mean_d = (Moh.T @ vals) / np.maximum(cam_cnt, 1.0)[:, None]

    Dc = D_cam.astype(f32).copy()
    rng = np.arange(C)
    diag = np.diagonal(Dc).copy()
    diag_new = GAMMA * diag + (1.0 - GAMMA) * np.maximum(intra_mean, 1e-6)
    Dc[rng, rng] = np.where(intra_cnt >= 1.0, diag_new, diag)
    present = cam_cnt > 0
    off_mask = present[:, None] & present[None, :] & (proxy_cnt[None, :] > 0) & (~np.eye(C, dtype=bool))
    with np.errstate(invalid='ignore'):
        upd = GAMMA * Dc + (1.0 - GAMMA) * np.maximum(mean_d, 1e-6)
    Dc = np.where(off_mask, upd, Dc)
    Dc = np.maximum(Dc, 1e-6)

    dist_raw = np.maximum(1.0 - x @ x.T, 1e-8).astype(f32)
    pos_m = (targets[:, None] == targets[None, :]) & (cams[:, None] != cams[None, :])
    neg_m = targets[:, None] != targets[None, :]
    hard_pos = np.argmin(np.where(pos_m, dist_raw, np.inf), axis=1)
    hard_neg = np.argmax(np.where(neg_m, dist_raw, -np.inf), axis=1)
    idx = np.arange(Bsz)
    ic = cams
    scale_pos = np.clip(Dc[ic, ic] / Dc[ic, cams[hard_pos]], 0.1, 10.0)
    scale_neg = np.clip(Dc[ic, ic] / Dc[ic, cams[hard_neg]], 0.1, 10.0)
    d_pos = dist_raw[idx, hard_pos] * scale_pos
    d_neg = dist_raw[idx, hard_neg] * scale_neg
    trip = np.maximum(d_pos - d_neg + 0.2, 0.0)
    valid = (pos_m.sum(1) > 0) & (neg_m.sum(1) > 0)
    cnt = f32(valid.sum())
    ins = np.where(valid, trip, 0.0).sum(dtype=f32) / max(cnt, f32(1.0)) if cnt > 0 else f32(0.0)
    return f32(dis), f32(ins)


# ---------------------------------------------------------------- entry point
def kernel(**inputs):
    inp = np.asarray(inputs["inputs"], dtype=np.float32)
    targets = np.asarray(inputs["targets"]).astype(np.int64)
    cams = np.asarray(inputs["cams"]).astype(np.int64)
    proxy = np.asarray(inputs["proxy"], dtype=np.float32)
    pids = np.asarray(inputs["pids"]).astype(np.int64)
    cids = np.asarray(inputs["cids"]).astype(np.int64)
    D_cam = np.asarray(inputs["D_cam"], dtype=np.float32)

    x = inp / np.maximum(np.linalg.norm(inp, axis=1, keepdims=True), 1e-12)
    x = x.astype(np.float32)
    xP = np.ascontiguousarray(
        x.T.reshape(D // 128, 128, B).transpose(1, 0, 2)
        .reshape(128, (D // 128) * B).astype(ml_dtypes.bfloat16))

    percore, slot, offs, W, njt = _plan(cids)

    key = (W, njt, tuple(slot))
    if key not in _cache:
        _cache[key] = _build(W, njt, slot, offs)
    nc = _cache[key]

    in_maps = []
    nposh_all = []
    for m in range(M):
        im, nposh = _prep_core(m, percore, slot, offs, W, proxy, targets, cams, pids)
        im["xP"] = xP
        in_maps.append(im)
        nposh_all.append(nposh)

    res = run_bass_kernel_spmd(nc, in_maps, core_ids=list(range(M)))
    outs = res.results

    # ---------------- merge main loss ----------------
    npos = np.zeros(B, dtype=np.int64)
    pos_cands = [[] for _ in range(B)]
    neg_cands = []
    cam_cands = [[] for _ in range(NCAMS)]
    for m in range(M):
        nposh = nposh_all[m]
        npos += nposh[0:64].sum(axis=1) + nposh[64:128].sum(axis=1)
        po = outs[m]["o_pos"]  # (128, njt*8)
        pos_pack = np.concatenate([po[0:64], po[64:128]], axis=1)
        for i in range(B):
            v = pos_pack[i]
            pos_cands[i].append(v[v > -50.0])  # pen'd values are <= -9e4
        negm = outs[m]["o_neg"]
        neg_cands.append(np.concatenate([negm[0:64], negm[64:128]], axis=1))
        camv = outs[m]["o_cam"]
        for c in range(NCAMS):
            h, cw = (0, c) if c < 3 else (1, c - 3)
            cam_cands[c].append(camv[64 * h:64 * h + 64, cw * 8:cw * 8 + 8])

    lse_pos = np.full(B, -np.inf)
    mean_pos = np.zeros(B)
    for i in range(B):
        v = np.concatenate(pos_cands[i]).astype(np.float64)
        assert len(v) == npos[i], (i, len(v), npos[i])
        if len(v):
            t = v / TEMP
            tm = t.max()
            lse_pos[i] = tm + np.log(np.exp(t - tm).sum())
            mean_pos[i] = t.mean()

    allneg = np.concatenate(neg_cands, axis=1)
    top50 = -np.partition(-allneg, NUM_HARDS - 1, axis=1)[:, :NUM_HARDS]
    t = top50 / TEMP
    tm = t.max(axis=1)
    lse_neg = tm + np.log(np.sum(np.exp(t - tm[:, None]), axis=1))

    lse = np.logaddexp(lse_pos, lse_neg)
    row = np.where(npos > 0, lse - mean_pos, 0.0)
    loss = row.sum() / B

    # ---------------- batch terms ----------------
    present = np.zeros(MAX_CAMS)
    np.add.at(present, cams, 1.0)
    multi_cam = (present > 0).sum() >= 2

    if multi_cam:
        vals = np.full((B, MAX_CAMS), np.inf, dtype=np.float32)
        for c in range(NCAMS):
            cand = np.concatenate(cam_cands[c], axis=1)  # (64, M*8) d units
            top5 = -np.partition(-cand, NK - 1, axis=1)[:, :NK]
            vals[:, c] = np.maximum(1.0 - top5, 1e-8).mean(axis=1)
        cids_hist = np.zeros(MAX_CAMS)
        np.add.at(cids_hist, cids, 1.0)
        dis, ins = _host_batch_terms(x, targets, cams, cids_hist, vals, D_cam)
        loss = loss + LAM_DIS * float(dis) + LAM_INS * float(ins)

    return np.float32(loss)
